# revision 26
# baseline (speedup 1.0000x reference)
"""Trainium2 Bass kernel for the BiRNN LM problem.

Computation (per step t over SEQ=64):
    emb    = we[tok_t]                       [B=32, E=32]
    hidden = tanh([emb, hidden] @ i2h)       [B=32, H=16]
    out_t  = exp(hidden @ i2o)               [B=32, V=32000]
    out_t /= sum(out_t)                      (global sum over the whole slab)

Sharding: sequence dim across 8 cores, interleaved: core c produces output
steps {c + 8k, k=0..7}.  Each step's normalization sum is fully local to one
core => no collectives.  Every core replicates the (tiny) 64-step hidden
recurrence; per-core step selection uses partition_id()-based dynamic SBUF
slices, so all cores run one identical SPMD program.

Layout tricks:
  * EMBH[k] [48, 33*32]: rows 0:32 hold transposed embeddings for steps
    32k..32k+31 (col block t-32k), rows 32:48 hold transposed hiddens, with
    h_t landed at col block t+1-32k by the tanh itself.  The recurrence is
    then ONE matmul (lhsT=i2h [48,16]) + one tanh per step.
  * Output pass packs 4 steps x 32 batch onto the 128 partitions; exp runs
    on 1536-wide PSUM macros with fused accumulation; per-step sums and the
    1/S broadcast are two tiny mask matmuls (cross-partition ops on PE).

Pipeline order (v2): chain half 0 -> group 0 -> chain half 1 -> group 1,
so group 0's matmul+exp sweep starts right after 32 chain steps instead of
64, and group 0's HBM writes begin ~60us earlier.  Output writes stream on
the sync (SP) queue exclusively; group staging DMAs ride gpsimd/sync so
they never queue behind the 46us write bursts.
"""

import sys
import numpy as np

sys.path.insert(0, "/opt/trn_rl_repo")

import concourse.bass as bass
import concourse.bacc as bacc
import concourse.mybir as mybir
import concourse.tile as tile
from concourse.bass_types import DynSlice
from concourse.bass_utils import run_bass_kernel_spmd

F32 = mybir.dt.float32
F32R = mybir.dt.float32r
BF16 = mybir.dt.bfloat16
I32 = mybir.dt.int32
AF = mybir.ActivationFunctionType

SEQ, B, E, H, V = 64, 32, 32, 16, 32000
NCORES = 8
NGROUP = 2                   # groups of 4 steps (4*32 = 128 partitions)
GSTEP = 4
CHUNK = 512                  # matmul free dim (one PSUM bank; f32r hard limit)
MACRO = 1536                 # ACT exp granularity (3 banks)
PIECE = 3072                 # mul + DMA granularity (2 macros)
VQ = 8192                    # padded vocab quarter (i2o rows 32q..32q+16)
CHAIN_F32R = False            # single-pass PE matmuls for the recurrence


def build():
    nc = bacc.Bacc("TRN2", target_bir_lowering=False, debug=False,
                   num_devices=NCORES)

    tok_d = nc.dram_tensor("tokT", [128, 16], I32, kind="ExternalInput")
    h0_d = nc.dram_tensor("h0T", [H, B], F32, kind="ExternalInput")
    we_d = nc.dram_tensor("we", [V, E], F32, kind="ExternalInput")
    i2h_d = nc.dram_tensor("i2h", [E + H, H], F32, kind="ExternalInput")
    i2o_d = nc.dram_tensor("i2oQ", [64, VQ], F32R, kind="ExternalInput")
    mask_d = nc.dram_tensor("mask4", [128, 4], F32, kind="ExternalInput")
    maskT_d = nc.dram_tensor("maskT4", [4, 128], F32, kind="ExternalInput")

    out_d = nc.dram_tensor("out", [NGROUP, 128, V], F32, kind="ExternalOutput")
    # per-half hidden archives: h of step t at cols 32*(t-32k)
    harc = [nc.dram_tensor(f"harc{k}", [H, 32 * B], F32, kind="Internal")
            for k in range(2)]

    pieces = [min(PIECE, V - k * PIECE) for k in range((V + PIECE - 1) // PIECE)]
    # macros: (piece, col within piece, width); every macro inside one piece
    grid = []
    col = 0
    while col < V:
        piece = col // PIECE
        w = min(MACRO, V - col, (piece + 1) * PIECE - col)
        grid.append((piece, col - piece * PIECE, w))
        col += w
    nmacro = len(grid)

    with tile.TileContext(nc) as tc:
        with (
            tc.tile_pool(name="const", bufs=1) as constp,
            tc.tile_pool(name="embg", bufs=16) as embgp,
            tc.tile_pool(name="grp", bufs=2) as grpp,
            tc.tile_pool(name="slab", bufs=2 * len(pieces) - 2) as slabp,
            tc.tile_pool(name="stg", bufs=3) as stgp,
            tc.tile_pool(name="pmm", bufs=2, space="PSUM") as pmmp,
            tc.tile_pool(name="phc", bufs=1, space="PSUM") as phcp,
            tc.tile_pool(name="pmisc", bufs=1, space="PSUM") as pmiscp,
        ):
            pid = nc.partition_id(
                engines=(mybir.EngineType.SP, mybir.EngineType.Pool))

            # ---- constants / inputs to SBUF (sync queue; tok FIRST so the
            # gpsimd gather stream can start as early as possible) ----
            tok = constp.tile([128, 16], I32)
            nc.sync.dma_start(tok[:], tok_d.ap())
            i2h = constp.tile([E + H, H], F32)
            nc.sync.dma_start(i2h[:], i2h_d.ap())

            # combined [emb; h] per half: blocks 0..32, h_t stored at block t+1
            embh = [constp.tile([E + H, 33 * B], F32, name=f"embh{k}")
                    for k in range(2)]
            nc.sync.dma_start(embh[0][E:E + H, 0:B], h0_d.ap())
            mask4 = constp.tile([128, 4], F32)
            nc.sync.dma_start(mask4[:], mask_d.ap())
            maskT4 = constp.tile([4, 128], F32)
            nc.sync.dma_start(maskT4[:], maskT_d.ap())

            # warm the SP engine's dynamic-descriptor state (pid snap + harc
            # base-address register loads) off the critical path: the first
            # DynSlice DMA on a DRAM tensor otherwise pays ~2us of lazy
            # TENSOR_LOADs right between chain half 0 and group 0's sweep.
            warm = constp.tile([H, B], F32, name="warm")
            nc.sync.dma_start(warm[:], harc[0].ap()[:, DynSlice(pid * B, B)])
            nc.sync.dma_start(warm[:], harc[1].ap()[:, DynSlice(pid * B, B)])

            # ---- embedding gather + DVE 32x32 block transposes ----
            # i2o (2MB) rides the SAME gpsimd/Q0 queue, interleaved AFTER
            # chain half 0's gathers (j<8): on Q1 it starves the gather
            # descriptors of DMA engines exactly when the chain needs them
            # (measured 13us stall).  Quarter q still lands well before the
            # first matmul needing it (~macro 6q).
            i2o = constp.tile([128, VQ], F32R)
            i2o_after = {8: 0, 10: 1, 12: 2, 14: 3}
            for j in range(16):
                eg = embgp.tile([128, E], F32, tag="eg")
                nc.gpsimd.indirect_dma_start(
                    out=eg[:], out_offset=None, in_=we_d.ap(),
                    in_offset=bass.IndirectOffsetOnAxis(ap=tok[:, j:j + 1], axis=0))
                for b in range(4):
                    nc.vector.transpose(
                        embh[j // 8][0:E, 128 * (j % 8) + 32 * b:
                                     128 * (j % 8) + 32 * (b + 1)],
                        eg[32 * b:32 * (b + 1), :])
                if j in i2o_after:
                    q = i2o_after[j]
                    nc.gpsimd.dma_start(i2o[32 * q:32 * q + H, :],
                                        i2o_d.ap()[16 * q:16 * q + H, :])

            # ---- recurrence: one matmul + one tanh per step ----
            # f32r operands make each step ONE ldweights+matmul pair instead
            # of the 2-pass fp32 lowering (674ns -> ~340ns of PE per step).
            CD = F32R if CHAIN_F32R else F32

            def chain_step(t):
                k, b = t // 32, t % 32
                hp = phcp.tile([H, B], F32, space="PSUM", tag="hps")
                nc.tensor.matmul(hp[:], i2h[:].bitcast(CD),
                                 embh[k][:, B * b:B * (b + 1)].bitcast(CD),
                                 start=True, stop=True)
                dstk, dstb = (t + 1) // 32, (t + 1) % 32
                if t == 63:
                    dstk, dstb = 1, 32   # park h_63 in embh[1] block 32
                nc.scalar.activation(
                    embh[dstk][E:E + H, B * dstb:B * (dstb + 1)],
                    hp[:], AF.Tanh)

            def chain_steps(t0, t1):
                for t in range(t0, t1):
                    chain_step(t)

            def stage(g):
                # archive this half's hiddens to DRAM, then gather this
                # core's 4 steps back with dynamic-offset reads: h of step
                # c+32g+8i sits at archive col 32*(c+8i), c = partition_id.
                # g0 splits issue across sync+scalar queues (both idle right
                # after chain half 0); g1 stays on gpsimd AND is issued early
                # (from group 0's macro loop) so its DMA semaphore slots are
                # assigned BEFORE group 0's write burst -- late slots collide
                # with write transfers and stall the gather by ~30us.
                # both the sync and gpsimd queues are idle at staging time;
                # split the archive + 4 gather DMAs across them so the two
                # DRAM round trips overlap
                eng = nc.sync if g == 0 else nc.gpsimd
                eng2 = nc.sync if g == 0 else nc.sync
                if g == 0:
                    eng.dma_start(harc[0].ap()[:, 0:31 * B],
                                  embh[0][E:E + H, B:32 * B])
                    eng2.dma_start(harc[0].ap()[:, 31 * B:32 * B],
                                   embh[1][E:E + H, 0:B])
                else:
                    eng.dma_start(harc[1].ap()[:, 0:16 * B],
                                  embh[1][E:E + H, B:17 * B])
                    eng2.dma_start(harc[1].ap()[:, 16 * B:32 * B],
                                   embh[1][E:E + H, 17 * B:33 * B])
                lhsT = grpp.tile([128, 128], F32R, tag="lhsT")
                hsrc = harc[g].ap().bitcast(F32R).rearrange(
                    "h (i r) -> h i r", i=GSTEP)
                for q in range(4):
                    e = eng if q % 2 == 0 else eng2
                    e.dma_start(lhsT[32 * q:32 * q + H, :],
                                hsrc[0:H, :, DynSlice(pid * B, B)])
                return lhsT

            def group(g, lhsT, interleave=None):
                partials = grpp.tile([128, nmacro], F32, tag="part")
                # bf16 slabs: BOTH groups' exp results fit in SBUF at once, so
                # group 1's sweep never waits on group 0's write completions.
                # The scale pass converts to f32 in a small staging pool.
                slabs = [slabp.tile([128, w], BF16, tag="slab",
                                    name=f"slab_{g}_{k}")
                        for k, w in enumerate(pieces)]
                for m, (piece, pcol, w) in enumerate(grid):
                    ps = pmmp.tile([128, MACRO], F32, space="PSUM", tag="mm")
                    c0 = 0
                    while c0 < w:
                        gcol = piece * PIECE + pcol + c0
                        q, qcol = gcol // VQ, gcol % VQ
                        cw = min(CHUNK, w - c0, VQ - qcol)
                        nc.tensor.matmul(
                            ps[:, c0:c0 + cw], lhsT[32 * q:32 * q + H, :],
                            i2o[32 * q:32 * q + H, qcol:qcol + cw],
                            start=True, stop=True,
                            tile_position=(32 * q, 0))
                        c0 += cw
                    if g == 0 and m < nmacro - 1:
                        nc.scalar.activation(
                            slabs[piece][:, pcol:pcol + w], ps[:, 0:w], AF.Exp)
                        nc.vector.tensor_reduce(partials[:, m:m + 1],
                                                slabs[piece][:, pcol:pcol + w],
                                                axis=mybir.AxisListType.X,
                                                op=mybir.AluOpType.add)
                    else:
                        nc.scalar.activation(
                            slabs[piece][:, pcol:pcol + w], ps[:, 0:w], AF.Exp,
                            accum_out=partials[:, m:m + 1])
                    if interleave is not None:
                        interleave(m)

                sums_ps = pmiscp.tile([4, nmacro], F32, space="PSUM", tag="misc")
                nc.tensor.matmul(sums_ps[:], mask4[:], partials[:],
                                 start=True, stop=True)
                s4 = grpp.tile([4, 1], F32, tag="s4")
                nc.vector.tensor_reduce(s4[:], sums_ps[:],
                                        axis=mybir.AxisListType.X,
                                        op=mybir.AluOpType.add)
                r4 = grpp.tile([4, 1], F32, tag="r4")
                nc.vector.reciprocal(r4[:], s4[:])
                bc_ps = pmiscp.tile([128, 1], F32, space="PSUM", tag="misc")
                nc.tensor.matmul(bc_ps[:], maskT4[:], r4[:], start=True, stop=True)
                scal = grpp.tile([128, 1], F32, tag="scal")
                nc.scalar.copy(scal[:], bc_ps[:])

                for k, w in enumerate(pieces):
                    stg = stgp.tile([128, w], F32, tag="stg")
                    nc.vector.tensor_scalar_mul(stg[:], slabs[k][:],
                                                scal[:, 0:1])
                    nc.sync.dma_start(out_d.ap()[g, :, PIECE * k:PIECE * k + w],
                                      stg[:])

            # v3 pipeline: group 0's compute starts right after chain half 0;
            # chain half 1 is interleaved INTO group 0's macro loop (2 steps
            # per exp macro on both the PE and ACT queues) so group 1 is
            # unblocked by the time group 0's writes are streaming.
            chain_steps(0, 32)

            # chain half 1 fully interleaved into group 0's macro loop (the
            # tanh mostly hides in the exp's pipelineable tail), so group 1's
            # staging can start the moment group 0's sweep ends.
            lhsT1_box = {}
            PRE = 15   # chain-1 steps run during group 0's staging window,
                       # where PE and ACT are idle: native 654ns pitch instead
                       # of the ~1.3us/step ACT-contention pitch inside the
                       # exp sweep.  Pulls tanh63 ~18us earlier so group 1's
                       # staging beats the write burst.

            def chain1_interleave(m):
                t0 = 32 + PRE + 2 * m
                for t in range(t0, min(t0 + 2, 64)):
                    chain_step(t)
                if t0 + 2 >= 64 and t0 < 64 + 2:
                    pass
                if m == 9:
                    lhsT1_box["lhsT"] = stage(1)

            lhsT0 = stage(0)
            chain_steps(32, 32 + PRE)
            group(0, lhsT0, interleave=chain1_interleave)
            group(1, lhsT1_box["lhsT"])

    nc.compile()
    return nc


_NC_CACHE = None


def _get_nc():
    global _NC_CACHE
    if _NC_CACHE is None:
        _NC_CACHE = build()
    return _NC_CACHE


def _prep_inputs(input_tokens, h0, we, i2h, i2o):
    flat = np.ascontiguousarray(input_tokens, dtype=np.int32).reshape(-1)  # (t,b)
    tokT = np.ascontiguousarray(flat.reshape(16, 128).T)                   # [128,16]
    h0T = np.ascontiguousarray(np.asarray(h0, np.float32).T)               # [16,32]
    we = np.ascontiguousarray(np.asarray(we, np.float32))
    i2h = np.ascontiguousarray(np.asarray(i2h, np.float32))
    i2o = np.asarray(i2o, np.float32)
    i2oQ = np.zeros((64, VQ), np.float32)
    for q in range(4):
        lo = VQ * q
        hi = min(lo + VQ, V)
        i2oQ[16 * q:16 * q + H, 0:hi - lo] = i2o[:, lo:hi]
    mask4 = np.zeros((128, 4), np.float32)
    mask4[np.arange(128), np.arange(128) // 32] = 1.0
    maskT4 = np.ascontiguousarray(mask4.T)
    shared = dict(tokT=tokT, h0T=h0T, we=we, i2h=i2h, i2oQ=i2oQ,
                  mask4=mask4, maskT4=maskT4)
    return [dict(shared) for _ in range(NCORES)]


def _assemble(results):
    full = np.empty((SEQ, B, V), np.float32)
    for c in range(NCORES):
        o = results[c]["out"].reshape(NGROUP, GSTEP, B, V)
        for g in range(NGROUP):
            for i in range(GSTEP):
                full[c + 32 * g + 8 * i] = o[g, i]
    return full


def run(inputs, trace=False, **kw):
    nc = _get_nc()
    in_maps = _prep_inputs(**inputs)
    res = run_bass_kernel_spmd(nc, in_maps, list(range(NCORES)), trace=trace, **kw)
    return _assemble(res.results), res


def kernel(**inputs):
    out, _ = run(inputs, trace=False)
    return out


# revision 27
# speedup vs baseline: 1.0260x; 1.0260x over previous
"""Trainium2 Bass kernel for the BiRNN LM problem.

Computation (per step t over SEQ=64):
    emb    = we[tok_t]                       [B=32, E=32]
    hidden = tanh([emb, hidden] @ i2h)       [B=32, H=16]
    out_t  = exp(hidden @ i2o)               [B=32, V=32000]
    out_t /= sum(out_t)                      (global sum over the whole slab)

Sharding: sequence dim across 8 cores, interleaved: core c produces output
steps {c + 8k, k=0..7}.  Each step's normalization sum is fully local to one
core => no collectives.  Every core replicates the (tiny) 64-step hidden
recurrence; per-core step selection uses partition_id()-based dynamic SBUF
slices, so all cores run one identical SPMD program.

Layout tricks:
  * EMBH[k] [48, 33*32]: rows 0:32 hold transposed embeddings for steps
    32k..32k+31 (col block t-32k), rows 32:48 hold transposed hiddens, with
    h_t landed at col block t+1-32k by the tanh itself.  The recurrence is
    then ONE matmul (lhsT=i2h [48,16]) + one tanh per step.
  * Output pass packs 4 steps x 32 batch onto the 128 partitions; exp runs
    on 1536-wide PSUM macros with fused accumulation; per-step sums and the
    1/S broadcast are two tiny mask matmuls (cross-partition ops on PE).

Pipeline order (v2): chain half 0 -> group 0 -> chain half 1 -> group 1,
so group 0's matmul+exp sweep starts right after 32 chain steps instead of
64, and group 0's HBM writes begin ~60us earlier.  Output writes stream on
the sync (SP) queue exclusively; group staging DMAs ride gpsimd/sync so
they never queue behind the 46us write bursts.
"""

import sys
import numpy as np

sys.path.insert(0, "/opt/trn_rl_repo")

import concourse.bass as bass
import concourse.bacc as bacc
import concourse.mybir as mybir
import concourse.tile as tile
from concourse.bass_types import DynSlice
from concourse.bass_utils import run_bass_kernel_spmd

F32 = mybir.dt.float32
F32R = mybir.dt.float32r
BF16 = mybir.dt.bfloat16
I32 = mybir.dt.int32
AF = mybir.ActivationFunctionType

SEQ, B, E, H, V = 64, 32, 32, 16, 32000
NCORES = 8
NGROUP = 2                   # groups of 4 steps (4*32 = 128 partitions)
GSTEP = 4
CHUNK = 512                  # matmul free dim (one PSUM bank; f32r hard limit)
MACRO = 1536                 # ACT exp granularity (3 banks)
PIECE = 3072                 # mul + DMA granularity (2 macros)
VQ = 8192                    # padded vocab quarter (i2o rows 32q..32q+16)
CHAIN_F32R = False            # single-pass PE matmuls for the recurrence


def build():
    nc = bacc.Bacc("TRN2", target_bir_lowering=False, debug=False,
                   num_devices=NCORES)

    tok_d = nc.dram_tensor("tokT", [128, 16], I32, kind="ExternalInput")
    h0_d = nc.dram_tensor("h0T", [H, B], F32, kind="ExternalInput")
    we_d = nc.dram_tensor("we", [V, E], F32, kind="ExternalInput")
    i2h_d = nc.dram_tensor("i2h", [E + H, H], F32, kind="ExternalInput")
    i2o_d = nc.dram_tensor("i2oQ", [64, VQ], F32R, kind="ExternalInput")
    mask_d = nc.dram_tensor("mask4", [128, 4], F32, kind="ExternalInput")
    maskT_d = nc.dram_tensor("maskT4", [4, 128], F32, kind="ExternalInput")

    out_d = nc.dram_tensor("out", [NGROUP, 128, V], BF16, kind="ExternalOutput")
    # per-half hidden archives: h of step t at cols 32*(t-32k)
    harc = [nc.dram_tensor(f"harc{k}", [H, 32 * B], F32, kind="Internal")
            for k in range(2)]

    pieces = [min(PIECE, V - k * PIECE) for k in range((V + PIECE - 1) // PIECE)]
    # macros: (piece, col within piece, width); every macro inside one piece
    grid = []
    col = 0
    while col < V:
        piece = col // PIECE
        w = min(MACRO, V - col, (piece + 1) * PIECE - col)
        grid.append((piece, col - piece * PIECE, w))
        col += w
    nmacro = len(grid)

    with tile.TileContext(nc) as tc:
        with (
            tc.tile_pool(name="const", bufs=1) as constp,
            tc.tile_pool(name="embg", bufs=16) as embgp,
            tc.tile_pool(name="grp", bufs=2) as grpp,
            tc.tile_pool(name="slab", bufs=2 * len(pieces) - 2) as slabp,
            tc.tile_pool(name="stg", bufs=3) as stgp,
            tc.tile_pool(name="pmm", bufs=2, space="PSUM") as pmmp,
            tc.tile_pool(name="phc", bufs=1, space="PSUM") as phcp,
            tc.tile_pool(name="pmisc", bufs=1, space="PSUM") as pmiscp,
        ):
            pid = nc.partition_id(
                engines=(mybir.EngineType.SP, mybir.EngineType.Pool))

            # ---- constants / inputs to SBUF (sync queue; tok FIRST so the
            # gpsimd gather stream can start as early as possible) ----
            tok = constp.tile([128, 16], I32)
            nc.sync.dma_start(tok[:], tok_d.ap())
            i2h = constp.tile([E + H, H], F32)
            nc.sync.dma_start(i2h[:], i2h_d.ap())

            # combined [emb; h] per half: blocks 0..32, h_t stored at block t+1
            embh = [constp.tile([E + H, 33 * B], F32, name=f"embh{k}")
                    for k in range(2)]
            nc.sync.dma_start(embh[0][E:E + H, 0:B], h0_d.ap())
            mask4 = constp.tile([128, 4], F32)
            nc.sync.dma_start(mask4[:], mask_d.ap())
            maskT4 = constp.tile([4, 128], F32)
            nc.sync.dma_start(maskT4[:], maskT_d.ap())

            # warm the SP engine's dynamic-descriptor state (pid snap + harc
            # base-address register loads) off the critical path: the first
            # DynSlice DMA on a DRAM tensor otherwise pays ~2us of lazy
            # TENSOR_LOADs right between chain half 0 and group 0's sweep.
            warm = constp.tile([H, B], F32, name="warm")
            nc.sync.dma_start(warm[:], harc[0].ap()[:, DynSlice(pid * B, B)])
            nc.sync.dma_start(warm[:], harc[1].ap()[:, DynSlice(pid * B, B)])

            # ---- embedding gather + DVE 32x32 block transposes ----
            # i2o (2MB) rides the SAME gpsimd/Q0 queue, interleaved AFTER
            # chain half 0's gathers (j<8): on Q1 it starves the gather
            # descriptors of DMA engines exactly when the chain needs them
            # (measured 13us stall).  Quarter q still lands well before the
            # first matmul needing it (~macro 6q).
            i2o = constp.tile([128, VQ], F32R)
            i2o_after = {8: 0, 10: 1, 12: 2, 14: 3}
            for j in range(16):
                eg = embgp.tile([128, E], F32, tag="eg")
                nc.gpsimd.indirect_dma_start(
                    out=eg[:], out_offset=None, in_=we_d.ap(),
                    in_offset=bass.IndirectOffsetOnAxis(ap=tok[:, j:j + 1], axis=0))
                for b in range(4):
                    nc.vector.transpose(
                        embh[j // 8][0:E, 128 * (j % 8) + 32 * b:
                                     128 * (j % 8) + 32 * (b + 1)],
                        eg[32 * b:32 * (b + 1), :])
                if j in i2o_after:
                    q = i2o_after[j]
                    nc.gpsimd.dma_start(i2o[32 * q:32 * q + H, :],
                                        i2o_d.ap()[16 * q:16 * q + H, :])

            # ---- recurrence: one matmul + one tanh per step ----
            # f32r operands make each step ONE ldweights+matmul pair instead
            # of the 2-pass fp32 lowering (674ns -> ~340ns of PE per step).
            CD = F32R if CHAIN_F32R else F32

            def chain_step(t):
                k, b = t // 32, t % 32
                hp = phcp.tile([H, B], F32, space="PSUM", tag="hps")
                nc.tensor.matmul(hp[:], i2h[:].bitcast(CD),
                                 embh[k][:, B * b:B * (b + 1)].bitcast(CD),
                                 start=True, stop=True)
                dstk, dstb = (t + 1) // 32, (t + 1) % 32
                if t == 63:
                    dstk, dstb = 1, 32   # park h_63 in embh[1] block 32
                nc.scalar.activation(
                    embh[dstk][E:E + H, B * dstb:B * (dstb + 1)],
                    hp[:], AF.Tanh)

            def chain_steps(t0, t1):
                for t in range(t0, t1):
                    chain_step(t)

            def stage(g):
                # archive this half's hiddens to DRAM, then gather this
                # core's 4 steps back with dynamic-offset reads: h of step
                # c+32g+8i sits at archive col 32*(c+8i), c = partition_id.
                # g0 splits issue across sync+scalar queues (both idle right
                # after chain half 0); g1 stays on gpsimd AND is issued early
                # (from group 0's macro loop) so its DMA semaphore slots are
                # assigned BEFORE group 0's write burst -- late slots collide
                # with write transfers and stall the gather by ~30us.
                # both the sync and gpsimd queues are idle at staging time;
                # split the archive + 4 gather DMAs across them so the two
                # DRAM round trips overlap
                eng = nc.sync if g == 0 else nc.gpsimd
                eng2 = nc.sync if g == 0 else nc.sync
                if g == 0:
                    eng.dma_start(harc[0].ap()[:, 0:31 * B],
                                  embh[0][E:E + H, B:32 * B])
                    eng2.dma_start(harc[0].ap()[:, 31 * B:32 * B],
                                   embh[1][E:E + H, 0:B])
                else:
                    eng.dma_start(harc[1].ap()[:, 0:16 * B],
                                  embh[1][E:E + H, B:17 * B])
                    eng2.dma_start(harc[1].ap()[:, 16 * B:32 * B],
                                   embh[1][E:E + H, 17 * B:33 * B])
                lhsT = grpp.tile([128, 128], F32R, tag="lhsT")
                hsrc = harc[g].ap().bitcast(F32R).rearrange(
                    "h (i r) -> h i r", i=GSTEP)
                for q in range(4):
                    e = eng if q % 2 == 0 else eng2
                    e.dma_start(lhsT[32 * q:32 * q + H, :],
                                hsrc[0:H, :, DynSlice(pid * B, B)])
                return lhsT

            def group(g, lhsT, interleave=None):
                partials = grpp.tile([128, nmacro], F32, tag="part")
                # bf16 slabs: BOTH groups' exp results fit in SBUF at once, so
                # group 1's sweep never waits on group 0's write completions.
                # The scale pass converts to f32 in a small staging pool.
                slabs = [slabp.tile([128, w], BF16, tag="slab",
                                    name=f"slab_{g}_{k}")
                        for k, w in enumerate(pieces)]
                for m, (piece, pcol, w) in enumerate(grid):
                    ps = pmmp.tile([128, MACRO], F32, space="PSUM", tag="mm")
                    c0 = 0
                    while c0 < w:
                        gcol = piece * PIECE + pcol + c0
                        q, qcol = gcol // VQ, gcol % VQ
                        cw = min(CHUNK, w - c0, VQ - qcol)
                        nc.tensor.matmul(
                            ps[:, c0:c0 + cw], lhsT[32 * q:32 * q + H, :],
                            i2o[32 * q:32 * q + H, qcol:qcol + cw],
                            start=True, stop=True,
                            tile_position=(32 * q, 0))
                        c0 += cw
                    if g == 0 and m < nmacro - 1:
                        nc.scalar.activation(
                            slabs[piece][:, pcol:pcol + w], ps[:, 0:w], AF.Exp)
                        nc.vector.tensor_reduce(partials[:, m:m + 1],
                                                slabs[piece][:, pcol:pcol + w],
                                                axis=mybir.AxisListType.X,
                                                op=mybir.AluOpType.add)
                    else:
                        nc.scalar.activation(
                            slabs[piece][:, pcol:pcol + w], ps[:, 0:w], AF.Exp,
                            accum_out=partials[:, m:m + 1])
                    if interleave is not None:
                        interleave(m)

                sums_ps = pmiscp.tile([4, nmacro], F32, space="PSUM", tag="misc")
                nc.tensor.matmul(sums_ps[:], mask4[:], partials[:],
                                 start=True, stop=True)
                s4 = grpp.tile([4, 1], F32, tag="s4")
                nc.vector.tensor_reduce(s4[:], sums_ps[:],
                                        axis=mybir.AxisListType.X,
                                        op=mybir.AluOpType.add)
                r4 = grpp.tile([4, 1], F32, tag="r4")
                nc.vector.reciprocal(r4[:], s4[:])
                bc_ps = pmiscp.tile([128, 1], F32, space="PSUM", tag="misc")
                nc.tensor.matmul(bc_ps[:], maskT4[:], r4[:], start=True, stop=True)
                scal = grpp.tile([128, 1], F32, tag="scal")
                nc.scalar.copy(scal[:], bc_ps[:])

                for k, w in enumerate(pieces):
                    stg = stgp.tile([128, w], BF16, tag="stg")
                    nc.vector.tensor_scalar_mul(stg[:], slabs[k][:],
                                                scal[:, 0:1])
                    nc.sync.dma_start(out_d.ap()[g, :, PIECE * k:PIECE * k + w],
                                      stg[:])

            # v3 pipeline: group 0's compute starts right after chain half 0;
            # chain half 1 is interleaved INTO group 0's macro loop (2 steps
            # per exp macro on both the PE and ACT queues) so group 1 is
            # unblocked by the time group 0's writes are streaming.
            chain_steps(0, 32)

            # chain half 1 fully interleaved into group 0's macro loop (the
            # tanh mostly hides in the exp's pipelineable tail), so group 1's
            # staging can start the moment group 0's sweep ends.
            lhsT1_box = {}
            PRE = 15   # chain-1 steps run during group 0's staging window,
                       # where PE and ACT are idle: native 654ns pitch instead
                       # of the ~1.3us/step ACT-contention pitch inside the
                       # exp sweep.  Pulls tanh63 ~18us earlier so group 1's
                       # staging beats the write burst.

            def chain1_interleave(m):
                t0 = 32 + PRE + 2 * m
                for t in range(t0, min(t0 + 2, 64)):
                    chain_step(t)
                if t0 + 2 >= 64 and t0 < 64 + 2:
                    pass
                if m == 9:
                    lhsT1_box["lhsT"] = stage(1)

            lhsT0 = stage(0)
            chain_steps(32, 32 + PRE)
            group(0, lhsT0, interleave=chain1_interleave)
            group(1, lhsT1_box["lhsT"])

    nc.compile()
    return nc


_NC_CACHE = None


def _get_nc():
    global _NC_CACHE
    if _NC_CACHE is None:
        _NC_CACHE = build()
    return _NC_CACHE


def _prep_inputs(input_tokens, h0, we, i2h, i2o):
    flat = np.ascontiguousarray(input_tokens, dtype=np.int32).reshape(-1)  # (t,b)
    tokT = np.ascontiguousarray(flat.reshape(16, 128).T)                   # [128,16]
    h0T = np.ascontiguousarray(np.asarray(h0, np.float32).T)               # [16,32]
    we = np.ascontiguousarray(np.asarray(we, np.float32))
    i2h = np.ascontiguousarray(np.asarray(i2h, np.float32))
    i2o = np.asarray(i2o, np.float32)
    i2oQ = np.zeros((64, VQ), np.float32)
    for q in range(4):
        lo = VQ * q
        hi = min(lo + VQ, V)
        i2oQ[16 * q:16 * q + H, 0:hi - lo] = i2o[:, lo:hi]
    mask4 = np.zeros((128, 4), np.float32)
    mask4[np.arange(128), np.arange(128) // 32] = 1.0
    maskT4 = np.ascontiguousarray(mask4.T)
    shared = dict(tokT=tokT, h0T=h0T, we=we, i2h=i2h, i2oQ=i2oQ,
                  mask4=mask4, maskT4=maskT4)
    return [dict(shared) for _ in range(NCORES)]


def _assemble(results):
    full = np.empty((SEQ, B, V), np.float32)
    for c in range(NCORES):
        o = np.asarray(results[c]["out"]).astype(np.float32)
        o = o.reshape(NGROUP, GSTEP, B, V)
        for g in range(NGROUP):
            for i in range(GSTEP):
                full[c + 32 * g + 8 * i] = o[g, i]
    return full


def run(inputs, trace=False, **kw):
    nc = _get_nc()
    in_maps = _prep_inputs(**inputs)
    res = run_bass_kernel_spmd(nc, in_maps, list(range(NCORES)), trace=trace, **kw)
    return _assemble(res.results), res


def kernel(**inputs):
    out, _ = run(inputs, trace=False)
    return out


# revision 28
# speedup vs baseline: 1.1769x; 1.1471x over previous
"""Trainium2 Bass kernel for the BiRNN LM problem.

Computation (per step t over SEQ=64):
    emb    = we[tok_t]                       [B=32, E=32]
    hidden = tanh([emb, hidden] @ i2h)       [B=32, H=16]
    out_t  = exp(hidden @ i2o)               [B=32, V=32000]
    out_t /= sum(out_t)                      (global sum over the whole slab)

Sharding: sequence dim across 8 cores, interleaved: core c produces output
steps {c + 8k, k=0..7}.  Each step's normalization sum is fully local to one
core => no collectives.  Every core replicates the (tiny) 64-step hidden
recurrence; per-core step selection uses partition_id()-based dynamic SBUF
slices, so all cores run one identical SPMD program.

Layout tricks:
  * EMBH[k] [48, 33*32]: rows 0:32 hold transposed embeddings for steps
    32k..32k+31 (col block t-32k), rows 32:48 hold transposed hiddens, with
    h_t landed at col block t+1-32k by the tanh itself.  The recurrence is
    then ONE matmul (lhsT=i2h [48,16]) + one tanh per step.
  * Output pass packs 4 steps x 32 batch onto the 128 partitions; exp runs
    on 1536-wide PSUM macros with fused accumulation; per-step sums and the
    1/S broadcast are two tiny mask matmuls (cross-partition ops on PE).

Pipeline order (v2): chain half 0 -> group 0 -> chain half 1 -> group 1,
so group 0's matmul+exp sweep starts right after 32 chain steps instead of
64, and group 0's HBM writes begin ~60us earlier.  Output writes stream on
the sync (SP) queue exclusively; group staging DMAs ride gpsimd/sync so
they never queue behind the 46us write bursts.
"""

import sys
import numpy as np

sys.path.insert(0, "/opt/trn_rl_repo")

import concourse.bass as bass
import concourse.bacc as bacc
import concourse.mybir as mybir
import concourse.tile as tile
from concourse.bass_types import DynSlice
from concourse.bass_utils import run_bass_kernel_spmd

F32 = mybir.dt.float32
F32R = mybir.dt.float32r
BF16 = mybir.dt.bfloat16
I32 = mybir.dt.int32
AF = mybir.ActivationFunctionType

SEQ, B, E, H, V = 64, 32, 32, 16, 32000
NCORES = 8
NGROUP = 2                   # groups of 4 steps (4*32 = 128 partitions)
GSTEP = 4
CHUNK = 512                  # matmul free dim (one PSUM bank; f32r hard limit)
MACRO = 1536                 # ACT exp granularity (3 banks)
PIECE = 3072                 # mul + DMA granularity (2 macros)
VQ = 8192                    # padded vocab quarter (i2o rows 32q..32q+16)
CHAIN_F32R = False            # single-pass PE matmuls for the recurrence


def build():
    nc = bacc.Bacc("TRN2", target_bir_lowering=False, debug=False,
                   num_devices=NCORES)

    tok_d = nc.dram_tensor("tokT", [128, 16], I32, kind="ExternalInput")
    h0_d = nc.dram_tensor("h0T", [H, B], F32, kind="ExternalInput")
    we_d = nc.dram_tensor("we", [V, E], F32, kind="ExternalInput")
    i2h_d = nc.dram_tensor("i2h", [E + H, H], F32, kind="ExternalInput")
    i2o_d = nc.dram_tensor("i2oQ", [64, VQ], F32R, kind="ExternalInput")
    mask_d = nc.dram_tensor("mask4", [128, 4], F32, kind="ExternalInput")
    maskT_d = nc.dram_tensor("maskT4", [4, 128], F32, kind="ExternalInput")

    out_d = nc.dram_tensor("out", [NGROUP, 128, V], BF16, kind="ExternalOutput")
    # per-half hidden archives: h of step t at cols 32*(t-32k)
    harc = [nc.dram_tensor(f"harc{k}", [H, 32 * B], F32, kind="Internal")
            for k in range(2)]

    pieces = [min(PIECE, V - k * PIECE) for k in range((V + PIECE - 1) // PIECE)]
    # macros: (piece, col within piece, width); every macro inside one piece
    grid = []
    col = 0
    while col < V:
        piece = col // PIECE
        w = min(MACRO, V - col, (piece + 1) * PIECE - col)
        grid.append((piece, col - piece * PIECE, w))
        col += w
    nmacro = len(grid)

    with tile.TileContext(nc) as tc:
        with (
            tc.tile_pool(name="const", bufs=1) as constp,
            tc.tile_pool(name="embg", bufs=16) as embgp,
            tc.tile_pool(name="grp", bufs=2) as grpp,
            tc.tile_pool(name="slab", bufs=2 * len(pieces) - 2) as slabp,
            tc.tile_pool(name="stg", bufs=3) as stgp,
            tc.tile_pool(name="pmm", bufs=2, space="PSUM") as pmmp,
            tc.tile_pool(name="phc", bufs=1, space="PSUM") as phcp,
            tc.tile_pool(name="pmisc", bufs=1, space="PSUM") as pmiscp,
        ):
            pid = nc.partition_id(
                engines=(mybir.EngineType.SP, mybir.EngineType.Pool))

            # ---- constants / inputs to SBUF (sync queue; tok FIRST so the
            # gpsimd gather stream can start as early as possible) ----
            tok = constp.tile([128, 16], I32)
            nc.sync.dma_start(tok[:], tok_d.ap())
            i2h = constp.tile([E + H, H], F32)
            nc.sync.dma_start(i2h[:], i2h_d.ap())

            # combined [emb; h] per half: blocks 0..32, h_t stored at block t+1
            embh = [constp.tile([E + H, 33 * B], F32, name=f"embh{k}")
                    for k in range(2)]
            nc.sync.dma_start(embh[0][E:E + H, 0:B], h0_d.ap())
            mask4 = constp.tile([128, 4], F32)
            nc.sync.dma_start(mask4[:], mask_d.ap())
            maskT4 = constp.tile([4, 128], F32)
            nc.sync.dma_start(maskT4[:], maskT_d.ap())

            # warm the SP engine's dynamic-descriptor state (pid snap + harc
            # base-address register loads) off the critical path: the first
            # DynSlice DMA on a DRAM tensor otherwise pays ~2us of lazy
            # TENSOR_LOADs right between chain half 0 and group 0's sweep.
            warm = constp.tile([H, B], F32, name="warm")
            nc.sync.dma_start(warm[:], harc[0].ap()[:, DynSlice(pid * B, B)])
            nc.sync.dma_start(warm[:], harc[1].ap()[:, DynSlice(pid * B, B)])

            # ---- embedding gather + DVE 32x32 block transposes ----
            # i2o (2MB) rides the SAME gpsimd/Q0 queue, interleaved AFTER
            # chain half 0's gathers (j<8): on Q1 it starves the gather
            # descriptors of DMA engines exactly when the chain needs them
            # (measured 13us stall).  Quarter q still lands well before the
            # first matmul needing it (~macro 6q).
            i2o = constp.tile([128, VQ], F32R)
            i2o_after = {8: 0, 10: 1, 12: 2, 14: 3}
            for j in range(16):
                eg = embgp.tile([128, E], F32, tag="eg")
                nc.gpsimd.indirect_dma_start(
                    out=eg[:], out_offset=None, in_=we_d.ap(),
                    in_offset=bass.IndirectOffsetOnAxis(ap=tok[:, j:j + 1], axis=0))
                for b in range(4):
                    nc.vector.transpose(
                        embh[j // 8][0:E, 128 * (j % 8) + 32 * b:
                                     128 * (j % 8) + 32 * (b + 1)],
                        eg[32 * b:32 * (b + 1), :])
                if j in i2o_after:
                    q = i2o_after[j]
                    nc.gpsimd.dma_start(i2o[32 * q:32 * q + H, :],
                                        i2o_d.ap()[16 * q:16 * q + H, :])

            # ---- recurrence: one matmul + one tanh per step ----
            # f32r operands make each step ONE ldweights+matmul pair instead
            # of the 2-pass fp32 lowering (674ns -> ~340ns of PE per step).
            CD = F32R if CHAIN_F32R else F32

            def chain_step(t):
                k, b = t // 32, t % 32
                hp = phcp.tile([H, B], F32, space="PSUM", tag="hps")
                nc.tensor.matmul(hp[:], i2h[:].bitcast(CD),
                                 embh[k][:, B * b:B * (b + 1)].bitcast(CD),
                                 start=True, stop=True)
                dstk, dstb = (t + 1) // 32, (t + 1) % 32
                if t == 63:
                    dstk, dstb = 1, 32   # park h_63 in embh[1] block 32
                nc.scalar.activation(
                    embh[dstk][E:E + H, B * dstb:B * (dstb + 1)],
                    hp[:], AF.Tanh)

            def chain_steps(t0, t1):
                for t in range(t0, t1):
                    chain_step(t)

            def stage(g):
                # archive this half's hiddens to DRAM, then gather this
                # core's 4 steps back with dynamic-offset reads: h of step
                # c+32g+8i sits at archive col 32*(c+8i), c = partition_id.
                # g0 splits issue across sync+scalar queues (both idle right
                # after chain half 0); g1 stays on gpsimd AND is issued early
                # (from group 0's macro loop) so its DMA semaphore slots are
                # assigned BEFORE group 0's write burst -- late slots collide
                # with write transfers and stall the gather by ~30us.
                # both the sync and gpsimd queues are idle at staging time;
                # split the archive + 4 gather DMAs across them so the two
                # DRAM round trips overlap
                eng = nc.sync if g == 0 else nc.gpsimd
                eng2 = nc.sync if g == 0 else nc.sync
                if g == 0:
                    eng.dma_start(harc[0].ap()[:, 0:31 * B],
                                  embh[0][E:E + H, B:32 * B])
                    eng2.dma_start(harc[0].ap()[:, 31 * B:32 * B],
                                   embh[1][E:E + H, 0:B])
                else:
                    eng.dma_start(harc[1].ap()[:, 0:16 * B],
                                  embh[1][E:E + H, B:17 * B])
                    eng2.dma_start(harc[1].ap()[:, 16 * B:32 * B],
                                   embh[1][E:E + H, 17 * B:33 * B])
                lhsT = grpp.tile([128, 128], F32R, tag="lhsT")
                hsrc = harc[g].ap().bitcast(F32R).rearrange(
                    "h (i r) -> h i r", i=GSTEP)
                for q in range(4):
                    e = eng if q % 2 == 0 else eng2
                    e.dma_start(lhsT[32 * q:32 * q + H, :],
                                hsrc[0:H, :, DynSlice(pid * B, B)])
                return lhsT

            def group(g, lhsT, interleave=None):
                partials = grpp.tile([128, nmacro], F32, tag="part")
                # bf16 slabs: BOTH groups' exp results fit in SBUF at once, so
                # group 1's sweep never waits on group 0's write completions.
                # The scale pass converts to f32 in a small staging pool.
                slabs = [slabp.tile([128, w], BF16, tag="slab",
                                    name=f"slab_{g}_{k}")
                        for k, w in enumerate(pieces)]
                for m, (piece, pcol, w) in enumerate(grid):
                    ps = pmmp.tile([128, MACRO], F32, space="PSUM", tag="mm")
                    c0 = 0
                    while c0 < w:
                        gcol = piece * PIECE + pcol + c0
                        q, qcol = gcol // VQ, gcol % VQ
                        cw = min(CHUNK, w - c0, VQ - qcol)
                        nc.tensor.matmul(
                            ps[:, c0:c0 + cw], lhsT[32 * q:32 * q + H, :],
                            i2o[32 * q:32 * q + H, qcol:qcol + cw],
                            start=True, stop=True,
                            tile_position=(32 * q, 0))
                        c0 += cw
                    if g == 0 and m < nmacro - 1:
                        nc.scalar.activation(
                            slabs[piece][:, pcol:pcol + w], ps[:, 0:w], AF.Exp)
                        nc.vector.tensor_reduce(partials[:, m:m + 1],
                                                slabs[piece][:, pcol:pcol + w],
                                                axis=mybir.AxisListType.X,
                                                op=mybir.AluOpType.add)
                    else:
                        nc.scalar.activation(
                            slabs[piece][:, pcol:pcol + w], ps[:, 0:w], AF.Exp,
                            accum_out=partials[:, m:m + 1])
                    if interleave is not None:
                        interleave(m)

                sums_ps = pmiscp.tile([4, nmacro], F32, space="PSUM", tag="misc")
                nc.tensor.matmul(sums_ps[:], mask4[:], partials[:],
                                 start=True, stop=True)
                s4 = grpp.tile([4, 1], F32, tag="s4")
                nc.vector.tensor_reduce(s4[:], sums_ps[:],
                                        axis=mybir.AxisListType.X,
                                        op=mybir.AluOpType.add)
                r4 = grpp.tile([4, 1], F32, tag="r4")
                nc.vector.reciprocal(r4[:], s4[:])
                bc_ps = pmiscp.tile([128, 1], F32, space="PSUM", tag="misc")
                nc.tensor.matmul(bc_ps[:], maskT4[:], r4[:], start=True, stop=True)
                scal = grpp.tile([128, 1], F32, tag="scal")
                nc.scalar.copy(scal[:], bc_ps[:])

                for k, w in enumerate(pieces):
                    stg = stgp.tile([128, w], BF16, tag="stg")
                    nc.vector.tensor_scalar_mul(stg[:], slabs[k][:],
                                                scal[:, 0:1])
                    nc.sync.dma_start(out_d.ap()[g, :, PIECE * k:PIECE * k + w],
                                      stg[:])

            # v3 pipeline: group 0's compute starts right after chain half 0;
            # chain half 1 is interleaved INTO group 0's macro loop (2 steps
            # per exp macro on both the PE and ACT queues) so group 1 is
            # unblocked by the time group 0's writes are streaming.
            chain_steps(0, 32)

            # chain half 1 fully interleaved into group 0's macro loop (the
            # tanh mostly hides in the exp's pipelineable tail), so group 1's
            # staging can start the moment group 0's sweep ends.
            lhsT1_box = {}
            PRE = 22   # chain-1 steps run during group 0's staging window,
                       # where PE and ACT are idle: native 654ns pitch instead
                       # of the ~1.3us/step ACT-contention pitch inside the
                       # exp sweep.  Pulls tanh63 ~18us earlier so group 1's
                       # staging beats the write burst.

            def chain1_interleave(m):
                t0 = 32 + PRE + 2 * m
                for t in range(t0, min(t0 + 2, 64)):
                    chain_step(t)
                if t0 + 2 >= 64 and t0 < 64 + 2:
                    pass
                if m == 5:
                    lhsT1_box["lhsT"] = stage(1)

            lhsT0 = stage(0)
            chain_steps(32, 32 + PRE)
            group(0, lhsT0, interleave=chain1_interleave)
            group(1, lhsT1_box["lhsT"])

    nc.compile()
    return nc


_NC_CACHE = None


def _get_nc():
    global _NC_CACHE
    if _NC_CACHE is None:
        _NC_CACHE = build()
    return _NC_CACHE


def _prep_inputs(input_tokens, h0, we, i2h, i2o):
    flat = np.ascontiguousarray(input_tokens, dtype=np.int32).reshape(-1)  # (t,b)
    tokT = np.ascontiguousarray(flat.reshape(16, 128).T)                   # [128,16]
    h0T = np.ascontiguousarray(np.asarray(h0, np.float32).T)               # [16,32]
    we = np.ascontiguousarray(np.asarray(we, np.float32))
    i2h = np.ascontiguousarray(np.asarray(i2h, np.float32))
    i2o = np.asarray(i2o, np.float32)
    i2oQ = np.zeros((64, VQ), np.float32)
    for q in range(4):
        lo = VQ * q
        hi = min(lo + VQ, V)
        i2oQ[16 * q:16 * q + H, 0:hi - lo] = i2o[:, lo:hi]
    mask4 = np.zeros((128, 4), np.float32)
    mask4[np.arange(128), np.arange(128) // 32] = 1.0
    maskT4 = np.ascontiguousarray(mask4.T)
    shared = dict(tokT=tokT, h0T=h0T, we=we, i2h=i2h, i2oQ=i2oQ,
                  mask4=mask4, maskT4=maskT4)
    return [dict(shared) for _ in range(NCORES)]


def _assemble(results):
    full = np.empty((SEQ, B, V), np.float32)
    for c in range(NCORES):
        o = np.asarray(results[c]["out"]).astype(np.float32)
        o = o.reshape(NGROUP, GSTEP, B, V)
        for g in range(NGROUP):
            for i in range(GSTEP):
                full[c + 32 * g + 8 * i] = o[g, i]
    return full


def run(inputs, trace=False, **kw):
    nc = _get_nc()
    in_maps = _prep_inputs(**inputs)
    res = run_bass_kernel_spmd(nc, in_maps, list(range(NCORES)), trace=trace, **kw)
    return _assemble(res.results), res


def kernel(**inputs):
    out, _ = run(inputs, trace=False)
    return out
